# revision 1
# baseline (speedup 1.0000x reference)
"""AdvancedSpikeEncoder Trainium2 Bass kernel.

Sharding: 8 cores, core c handles batch b=c//2, seq-half h=c%2 (128 seq rows).
On-device layout is "transposed world": partition dim = d-slice, free dim = s.
  d = C*128 + d'      (C in [0,4), d' in [0,128))   for [128, (C,s)=512] tiles
  d = j*128 + r*16 + d_sub, p = d_sub*8 + n          for the population layout
Population mean over n rides on TensorE (contraction over partitions with a
block-diagonal stationary); the weighted 4-encoder combine accumulates in one
PSUM bank per timestep via matmuls with (scaled-)identity stationaries.
"""

import math
import os
import sys

import numpy as np

for _p in ("/opt/trn_rl_repo", "/root/.axon_site/_ro/trn_rl_repo"):
    if os.path.isdir(_p) and _p not in sys.path:
        sys.path.insert(0, _p)

import ml_dtypes  # noqa: E402

import concourse.bass as bass  # noqa: E402
import concourse.tile as tile  # noqa: E402
from concourse import bacc, mybir  # noqa: E402
from concourse._compat import with_exitstack  # noqa: E402
from concourse.bass_utils import run_bass_kernel_spmd  # noqa: E402

B, T, S, D, N = 4, 16, 256, 512, 8
NCORES = 8
SH = 128  # seq rows per core
TWO_PI = float(2.0 * math.pi)

F32 = mybir.dt.float32
BF16 = mybir.dt.bfloat16
F16 = mybir.dt.float16
U8 = mybir.dt.uint8


@with_exitstack
def _body(ctx, tc, aps, w1, w3, add_pop_bias, out_dt):
    nc = tc.nc
    AF = mybir.ActivationFunctionType
    OP = mybir.AluOpType

    const = ctx.enter_context(tc.tile_pool(name="const", bufs=1))

    # ---------- constants / preamble ----------
    # All small constants arrive as two blobs (one f32, one bf16) so the
    # preamble is 2 DMAs instead of ~10; the named views below slice them.
    # f32 blob cols: xt 512 | nz 512 | fb 64 | sqb 16 | c25 1 | c223 1 |
    #                cm223 1 | [pb 32]
    ncf = 1107 + (32 if add_pop_bias else 0)
    cf = const.tile([128, ncf], F32, name="cf")
    nc.sync.dma_start(cf[:], aps["cfT"][:])
    xt = cf[:, 0:512]
    nz = cf[:, 512:1024]
    fb = cf[:, 1024:1088]
    sqb = cf[:, 1088:1104]
    c25 = cf[:, 1104:1105]
    c223 = cf[:, 1105:1106]
    cm223 = cf[:, 1106:1107]
    if add_pop_bias:
        pb = cf[:, 1107:1139]
    # bf16 blob cols: ew 1024 | w0I 128 | iI 128
    cb = const.tile([128, 1280], BF16, name="cb")
    nc.sync.dma_start(cb[:], aps["cbT"][:])
    ew = cb[:, 0:1024]
    wI = cb[:, 1024:1152]
    iI = cb[:, 1152:1280]
    # fp16 blob: xt1 512 | xt2 512 (x = xt1 + xt2 exactly to ~2^-22)
    ch = const.tile([128, 1024], F16, name="ch")
    nc.sync.dma_start(ch[:], aps["chT"][:])
    xt1 = ch[:, 0:512]
    xt2 = ch[:, 512:1024]

    sig = const.tile([128, 512], F32)
    nc.scalar.activation(sig[:], xt[:], AF.Sigmoid)
    tmp = const.tile([128, 512], F32)
    nc.scalar.activation(tmp[:], sig[:], AF.Copy, bias=0.05, scale=0.9)
    rn = const.tile([128, 512], F32)
    nc.vector.scalar_tensor_tensor(rn[:], nz[:], 0.1, tmp[:], OP.mult, OP.add)

    # work pool opens BEFORE the scoped pw pool: stack-allocating pw on top
    # means rp prefetch tiles never sit in pw's region (whose reuse would
    # serialize their DMAs behind the resp matmuls).
    work = ctx.enter_context(tc.tile_pool(name="work", bufs=2))

    # hoist a couple of t-step loads ahead of the pw chunks in the SP HWDGE
    # FIFO so the pw stream can't fully starve the t-loop prefetch
    pre_rp = {}

    def prefetch_rp(t):
        rp = work.tile([128, 4608], F32, tag="rp", bufs=5, name=f"rp_pre{t}")
        nc.sync.dma_start(rp[:], aps["rpT"][t])
        pre_rp[t] = rp

    # population response: resp[p=(d_sub,n), (j,s)] for r-bank = x @ pop_w.T
    # fp16 weights (fp32 PE streaming is 1/4 rate; fp16 halves the 8MB DMA
    # vs a bf16 split) with an exact fp16 two-term x: resp = P*(X1+X2) with
    # ~2e-4 std error from P's 11-bit mantissa -- negligible vs the draws.
    # kc-outer so each pw chunk is consumed as soon as its DMA lands.
    prates = const.tile([128, 4096], F32)
    with tc.tile_pool(name="psum_r", bufs=1, space="PSUM") as psum_r, \
         tc.tile_pool(name="pwpool", bufs=2) as pwpool:
        banks = [psum_r.tile([128, 512], F32, name=f"bank{r}") for r in range(8)]
        for kc in range(4):
            p1k = pwpool.tile([128, 4096], F16, tag="p1k")
            nc.sync.dma_start(p1k[:], aps["pw1T"][:, kc * 4096:(kc + 1) * 4096])
            # interleave one rp prefetch per chunk: keeps DMA busy through the
            # (PE-paced) resp phase without delaying the pw chunks themselves
            prefetch_rp(kc)
            for r in range(8):
                for j in range(4):
                    co = (r * 4 + j) * 128
                    rhs1 = xt1[:, kc * 128:(kc + 1) * 128]
                    rhs2 = xt2[:, kc * 128:(kc + 1) * 128]
                    out = banks[r][:, j * 128:(j + 1) * 128]
                    nc.tensor.matmul(out, p1k[:, co:co + 128], rhs1,
                                     start=(kc == 0 and j == 0), stop=False)
                    nc.tensor.matmul(out, p1k[:, co:co + 128], rhs2,
                                     start=False, stop=(kc == 3 and j == 3))
        for r in range(8):
            if add_pop_bias:
                for j in range(4):
                    nc.scalar.activation(
                        prates[:, r * 512 + j * 128:r * 512 + (j + 1) * 128],
                        banks[r][:, j * 128:(j + 1) * 128],
                        AF.Sigmoid, bias=pb[:, r * 4 + j:r * 4 + j + 1],
                    )
            else:
                nc.scalar.activation(prates[:, r * 512:(r + 1) * 512], banks[r][:], AF.Sigmoid)

    psum_o = ctx.enter_context(tc.tile_pool(name="psum_o", bufs=6, space="PSUM"))

    # DVE idles through the (PE/DMA-paced) resp phase; precompute the
    # sig-only temporal+phase encoders for the LAST timesteps there so the
    # end-of-kernel DVE backlog shrinks to just the two compares per step.
    PREENC = 9  # t >= PREENC precomputed

    def temporal_phase(t, sfx=""):
        """sig-only encoders for timestep t -> (oh, ph) bf16 tiles."""
        # temporal: one_hot(floor(15*sig))[t] == (15*sig - t - 0.5)^2 < 0.25
        sq = work.tile([128, 512], F32, tag="sq", name=f"sq{t}{sfx}")
        nc.scalar.activation(sq[:], sig[:], AF.Square, bias=sqb[:, t:t + 1], scale=15.0)
        oh = work.tile([128, 512], BF16, tag=f"oh{sfx}{t if sfx else ''}",
                       bufs=1 if sfx else 2, name=f"oh{t}{sfx}")
        nc.vector.tensor_scalar(oh[:], sq[:], 0.25, w1, OP.is_lt, OP.mult)

        # phase: sin(freq*t_val + 2*pi*sig) > 0.5 <=> frac(u) in (1/12, 5/12)
        # with u = sig + freq*t_val/(2*pi); round-to-nearest reduction is
        # valid because the window lies inside [0, 1/2).
        u = work.tile([128, 512], F32, tag="u", name=f"u{t}{sfx}")
        for c in range(4):
            nc.scalar.activation(
                u[:, c * 128:(c + 1) * 128], sig[:, c * 128:(c + 1) * 128],
                AF.Identity, bias=fb[:, t * 4 + c:t * 4 + c + 1],
            )
        v = work.tile([128, 512], F32, tag="v", name=f"v{t}{sfx}")
        nc.vector.tensor_scalar(v[:], u[:], 8388608.0, 8388608.0, OP.add, OP.subtract)
        fr = work.tile([128, 512], F32, tag="fr", name=f"fr{t}{sfx}")
        nc.vector.tensor_tensor(fr[:], u[:], v[:], OP.subtract)
        nc.scalar.activation(v[:], fr[:], AF.Square, bias=c25[:])
        ph = work.tile([128, 512], BF16, tag=f"ph{sfx}{t if sfx else ''}",
                       bufs=1 if sfx else 2, name=f"ph{t}{sfx}")
        nc.vector.tensor_scalar(ph[:], v[:], 1.0 / 36.0, w3, OP.is_lt, OP.mult)
        return oh, ph

    def encoders(t, rp, pre=None):
        """Full encoder set for timestep t -> (cpop, crate, oh, ph)."""
        cpop = work.tile([128, 4096], BF16, tag="cpop", bufs=2, name=f"cpop{t}")
        nc.vector.tensor_tensor(cpop[:], rp[:, 0:4096], prates[:], OP.is_lt)
        crate = work.tile([128, 512], BF16, tag="crate", bufs=2, name=f"crate{t}")
        nc.vector.tensor_tensor(crate[:], rp[:, 4096:4608], rn[:], OP.is_lt)
        oh, ph = pre if pre is not None else temporal_phase(t)
        return cpop, crate, oh, ph

    def combine_store(t, enc):
        """Weighted 4-encoder combine in one PSUM bank, evacuate, store."""
        cpop, crate, oh, ph = enc
        O = psum_o.tile([128, 512], F32, tag="O", name=f"O{t}")
        nc.tensor.matmul(O[:], wI[:], crate[:], start=True, stop=False)
        nc.tensor.matmul(O[:], iI[:], oh[:], start=False, stop=False)
        nc.tensor.matmul(O[:], iI[:], ph[:], start=False, stop=False)
        for r in range(8):
            nc.tensor.matmul(
                O[:], ew[:, r * 128:(r + 1) * 128], cpop[:, r * 512:(r + 1) * 512],
                start=False, stop=(r == 7),
            )
        ot = work.tile([128, 512], out_dt, tag="ot", bufs=3, name=f"ot{t}")
        # with standard weights every output is k/32, k in [0,32]: ship as
        # uint8 = 32*out (exact; host divides back)
        nc.scalar.activation(ot[:], O[:], AF.Copy,
                             scale=32.0 if out_dt == U8 else 1.0)
        nc.scalar.dma_start(aps["outT"][t], ot[:])

    # ---------- per-timestep loop (fully unrolled) ----------
    # t15's compares are hoisted right after prates (its input is prefetched
    # first) and held in dedicated tiles; the tail then only runs its matmuls
    # and store, instead of a full DVE chain after the last DMA.
    PRESET = list(range(PREENC, T))
    pre_enc = {t: temporal_phase(t, sfx="P") for t in PRESET}
    for t in range(T - 1):
        if t in pre_rp:
            rp = pre_rp[t]
        else:
            rp = work.tile([128, 4608], F32, tag="rp", bufs=5)
            nc.sync.dma_start(rp[:], aps["rpT"][t])
        combine_store(t, encoders(t, rp, pre=pre_enc.get(t)))

    # final timestep in quarters: each 1.15MB load chunk is compared (and its
    # population matmuls run) while the next chunk streams, so the post-last-
    # byte drain is one quarter-compare + two matmuls + store instead of the
    # full chain
    tL = T - 1
    rp = work.tile([128, 4608], F32, tag="rp", bufs=5, name="rpL")
    cpop = work.tile([128, 4096], BF16, tag="cpop", bufs=2, name="cpopL")
    crate = work.tile([128, 512], BF16, tag="crate", bufs=2, name="crateL")
    ohL, phL = pre_enc[tL]
    O = psum_o.tile([128, 512], F32, tag="O", name="OL")
    nc.tensor.matmul(O[:], iI[:], ohL[:], start=True, stop=False)
    nc.tensor.matmul(O[:], iI[:], phL[:], start=False, stop=False)
    for q in range(4):
        lo = q * 1024
        nc.sync.dma_start(rp[:, lo:lo + 1024], aps["rpT"][tL, :, lo:lo + 1024])
        nc.vector.tensor_tensor(cpop[:, lo:lo + 1024], rp[:, lo:lo + 1024],
                                prates[:, lo:lo + 1024], OP.is_lt)
        for r in (2 * q, 2 * q + 1):
            nc.tensor.matmul(
                O[:], ew[:, r * 128:(r + 1) * 128], cpop[:, r * 512:(r + 1) * 512],
                start=False, stop=False)
    nc.sync.dma_start(rp[:, 4096:4608], aps["rpT"][tL, :, 4096:4608])
    nc.vector.tensor_tensor(crate[:], rp[:, 4096:4608], rn[:], OP.is_lt)
    nc.tensor.matmul(O[:], wI[:], crate[:], start=False, stop=True)
    ot = work.tile([128, 512], out_dt, tag="ot", bufs=3, name="otL")
    nc.scalar.activation(ot[:], O[:], AF.Copy,
                         scale=32.0 if out_dt == U8 else 1.0)
    nc.sync.dma_start(aps["outT"][tL], ot[:])


_CACHE = {}


def _out_is_k32(w):
    # outputs are sums of {w0,w1,w3}*{0,1} + w2*k/8; with the standard 0.25
    # weights every value is k/32, k in [0,32] -> exactly a scaled uint8
    return all(float(x) == 0.25 for x in w)


def _build(w, add_pop_bias):
    key = (tuple(float(x) for x in w), bool(add_pop_bias))
    if key in _CACHE:
        return _CACHE[key]
    out_dt = U8 if _out_is_k32(w) else F32
    nc = bacc.Bacc(
        "TRN2", target_bir_lowering=False, debug=False,
        enable_asserts=False, num_devices=NCORES,
    )
    aps = {}
    def di(name, shape, dt):
        aps[name] = nc.dram_tensor(name, shape, dt, kind="ExternalInput").ap()
    di("rpT", [T, 128, 4608], F32)
    di("pw1T", [128, 16384], F16)
    di("cfT", [128, 1107 + (32 if add_pop_bias else 0)], F32)
    di("cbT", [128, 1280], BF16)
    di("chT", [128, 1024], F16)
    aps["outT"] = nc.dram_tensor("outT", [T, 128, 512], out_dt, kind="ExternalOutput").ap()

    with tile.TileContext(nc) as tc:
        _body(tc, aps, float(w[1]), float(w[3]), add_pop_bias, out_dt)
    nc.compile()
    _CACHE[key] = nc
    return nc


# ---------- host-side layout prep ----------

def _prep_ds(a):
    # [128 s, 512 d] -> [128 d', (C,s)]
    return np.ascontiguousarray(a.reshape(128, 4, 128).transpose(2, 1, 0)).reshape(128, 512)


def _prep_rr(a):
    # [T, 128 s, 512 d] -> [T, 128 d', (C,s)]
    return np.ascontiguousarray(a.reshape(T, 128, 4, 128).transpose(0, 3, 2, 1)).reshape(T, 128, 512)


def _prep_rp(a):
    # [T, 128 s, 512 d, 8 n] -> [T, 128 p=(d_sub,n), 4096 (r,j,s)]
    a6 = a.reshape(T, 128, 4, 8, 16, 8)  # t, s, j, r, d_sub, n
    return np.ascontiguousarray(a6.transpose(0, 4, 5, 3, 2, 1)).reshape(T, 128, 4096)


def _prep_pw(pwm):
    # [4096 e, 512 k] -> [128 k', 16384 (kc, r, j, p)]
    a = pwm.reshape(4, 8, 128, 4, 128)  # j, r, p, kc, k'
    return np.ascontiguousarray(a.transpose(4, 3, 1, 0, 2)).reshape(128, 16384)


def softmax_w(enc_weights):
    e = np.exp(enc_weights - enc_weights.max(), dtype=np.float32)
    return (e / e.sum(dtype=np.float32)).astype(np.float32)


def build_in_maps(inputs, w):
    x = np.asarray(inputs["x"], np.float32)
    freq_bands = np.asarray(inputs["freq_bands"], np.float32)
    pop_w = np.asarray(inputs["pop_w"], np.float32)
    pop_b = np.asarray(inputs["pop_b"], np.float32)
    noise_rate = np.asarray(inputs["noise_rate"], np.float32)
    rand_rate = np.asarray(inputs["rand_rate"], np.float32)
    rand_pop = np.asarray(inputs["rand_pop"], np.float32)
    add_pop_bias = bool(np.any(pop_b != 0))

    # shared constant tensors; fp16 pop_w for the resp matmuls (11-bit
    # mantissa -> resp error ~2e-4 std, negligible vs the Bernoulli draws)
    pwT = _prep_pw(pop_w)
    pw1T = pwT.astype(np.float16)
    t_vals = np.linspace(0.0, 2.0 * math.pi, T)
    fq = freq_bands.reshape(4, 128).astype(np.float64)  # C, d'
    fb = (fq[None, :, :] * t_vals[:, None, None] / (2.0 * math.pi)).astype(np.float32)
    fbT = np.ascontiguousarray(fb.transpose(2, 0, 1)).reshape(128, 64)
    sqbT = np.broadcast_to(
        -(np.arange(T, dtype=np.float32) + np.float32(0.5)), (128, T)).copy()
    c25T = np.full((128, 1), -0.25, np.float32)
    # ewm[p, (r, m)] = w2/8 where m = r*16 + p//8  (block-diag, zero-padded to
    # full 128 stationary columns so every matmul writes the whole PSUM bank)
    ewm = np.zeros((128, 8, 128), np.float32)
    for r in range(8):
        for ds in range(16):
            ewm[ds * 8:(ds + 1) * 8, r, r * 16 + ds] = w[2] / 8.0
    ewT = ewm.reshape(128, 1024).astype(ml_dtypes.bfloat16)
    w0I = (np.eye(128, dtype=np.float32) * w[0]).astype(ml_dtypes.bfloat16)
    I1 = np.eye(128, dtype=np.float32).astype(ml_dtypes.bfloat16)
    if add_pop_bias:
        # pb[p, r*4+j] = pop_b[e= j*1024 + r*128 + p]
        pbT = np.ascontiguousarray(
            pop_b.reshape(4, 8, 128).transpose(2, 1, 0)).reshape(128, 32)

    in_maps = []
    for c in range(NCORES):
        b, h = c // 2, c % 2
        sl = slice(h * SH, (h + 1) * SH)
        xTc = _prep_ds(x[b, sl])
        x1Tc = xTc.astype(np.float16)
        x2Tc = (xTc - x1Tc.astype(np.float32)).astype(np.float16)
        cf_parts = [xTc, _prep_ds(noise_rate[b, sl]), fbT, sqbT, c25T,
                    np.full((128, 1), 8388608.0, np.float32),
                    np.full((128, 1), -8388608.0, np.float32)]
        if add_pop_bias:
            cf_parts.append(pbT)
        m = {
            "rpT": np.concatenate(
                [_prep_rp(rand_pop[b, :, sl]), _prep_rr(rand_rate[b, :, sl])],
                axis=2),
            "pw1T": pw1T,
            "cfT": np.ascontiguousarray(np.concatenate(cf_parts, axis=1)),
            "cbT": np.ascontiguousarray(np.concatenate([ewT, w0I, I1], axis=1)),
            "chT": np.ascontiguousarray(np.concatenate([x1Tc, x2Tc], axis=1)),
        }
        in_maps.append(m)
    return in_maps


def kernel(x, freq_bands, pop_w, pop_b, enc_weights, noise_rate, rand_rate, rand_pop):
    inputs = dict(x=x, freq_bands=freq_bands, pop_w=pop_w, pop_b=pop_b,
                  enc_weights=enc_weights, noise_rate=noise_rate,
                  rand_rate=rand_rate, rand_pop=rand_pop)
    w = softmax_w(np.asarray(enc_weights, np.float32))
    add_pop_bias = bool(np.any(np.asarray(pop_b) != 0))
    nc = _build(w, add_pop_bias)
    in_maps = build_in_maps(inputs, w)

    res = run_bass_kernel_spmd(nc, in_maps, core_ids=list(range(NCORES)))

    out = np.empty((B, T, S, D), np.float32)
    for c in range(NCORES):
        b, h = c // 2, c % 2
        o = res.results[c]["outT"]  # [T, 128 d', (C,s)]
        if o.dtype == np.uint8:
            o = o.astype(np.float32) * np.float32(1.0 / 32.0)
        else:
            o = np.asarray(o, np.float32)
        o = o.reshape(T, 128, 4, 128).transpose(0, 3, 2, 1).reshape(T, SH, D)
        out[b, :, h * SH:(h + 1) * SH, :] = o
    return out



# revision 5
# speedup vs baseline: 1.4320x; 1.4320x over previous
"""AdvancedSpikeEncoder Trainium2 Bass kernel.

Sharding: 8 cores, core c handles batch b=c//2, seq-half h=c%2 (128 seq rows).
On-device layout is "transposed world": partition dim = d-slice, free dim = s.
  d = C*128 + d'      (C in [0,4), d' in [0,128))   for [128, (C,s)=512] tiles
  d = j*128 + r*16 + d_sub, p = d_sub*8 + n          for the population layout

DMA-minimizing dtypes (the kernel is HBM-bound):
  - Bernoulli draws ship as uint16 fixed point (floor(65536*u)); thresholds
    are built on device as RN(65535*p) uint16, so each compare is one 2-byte
    DVE op at the 2x rate and the quantization flip rate is ~1e-5.
  - pop_w and x ship fp16 (resp error ~3e-4 std, negligible vs draws).
  - sigmoid(x) ships f32: temporal/phase windows are sensitive to sig error
    (fp16 sig alone costs ~7e-3 rel), the 0.26MB is worth it.
  - output ships u8 (all outputs are exact multiples of 1/32 for the
    standard softmax weights; host divides back).

Encoders per timestep:
  - pop+rate: one fused [128,4608] uint16 is_lt against [prates16|r16].
  - temporal: st = floor(15*sig) once (round-trick, magic 1.5*2^23), then
    one 4x-mode is_equal per t.
  - phase: sin(f*t_val + 2pi*sig) > 0.5  <=>  |frac(u) - 1/4| < 1/6 with
    u = sig + t*f/15. With q = (sig + FBF_t - 1)^2 (FBF_t = frac-reduced
    offset, ScalarE Square with per-partition bias) the window test is
    (q - 13/36)^2 > 1/9: two ScalarE Squares + one DVE compare per t.
The weighted combine accumulates in one PSUM bank per timestep via matmuls
with (scaled-)identity stationaries; population mean over n rides on the
same bank via a block-diagonal stationary.
"""

import math
import os
import sys

import numpy as np

for _p in ("/opt/trn_rl_repo", "/root/.axon_site/_ro/trn_rl_repo"):
    if os.path.isdir(_p) and _p not in sys.path:
        sys.path.insert(0, _p)

import ml_dtypes  # noqa: E402

import concourse.bass as bass  # noqa: E402,F401
import concourse.tile as tile  # noqa: E402
from concourse import bacc, mybir  # noqa: E402
from concourse._compat import with_exitstack  # noqa: E402
from concourse.bass_utils import run_bass_kernel_spmd  # noqa: E402

B, T, S, D, N = 4, 16, 256, 512, 8
NCORES = 8
SH = 128  # seq rows per core
TWO_PI = float(2.0 * math.pi)
MAGIC = 12582912.0  # 1.5 * 2^23: round-to-nearest-int trick valid for |y| < 2^22

F32 = mybir.dt.float32
BF16 = mybir.dt.bfloat16
F16 = mybir.dt.float16
U16 = mybir.dt.uint16
U8 = mybir.dt.uint8


@with_exitstack
def _body(ctx, tc, aps, w1, w3, add_pop_bias, out_dt):
    nc = tc.nc
    AF = mybir.ActivationFunctionType
    OP = mybir.AluOpType

    const = ctx.enter_context(tc.tile_pool(name="const", bufs=1))

    # ---------- constants / preamble ----------
    # f32 blob cols: sig 512 | nz 512 | fbq 64 | c1336 1 | [pb 32]
    ncf = 1089 + (32 if add_pop_bias else 0)
    cf = const.tile([128, ncf], F32, name="cf")
    nc.sync.dma_start(cf[:], aps["cfT"][:])
    sig = cf[:, 0:512]
    nz = cf[:, 512:1024]
    fbq = cf[:, 1024:1088]
    c1336 = cf[:, 1088:1089]
    if add_pop_bias:
        pb = cf[:, 1089:1121]
    # bf16 blob cols: ew 1024 | w0I 128 | iI 128
    cb = const.tile([128, 1280], BF16, name="cb")
    nc.sync.dma_start(cb[:], aps["cbT"][:])
    ew = cb[:, 0:1024]
    wI = cb[:, 1024:1152]
    iI = cb[:, 1152:1280]
    # fp16 blob: x 512 (for the resp matmul only)
    ch = const.tile([128, 512], F16, name="ch")
    nc.sync.dma_start(ch[:], aps["chT"][:])
    xh = ch[:, 0:512]

    # fused compare threshold [prates16 | r16], filled below
    thr = const.tile([128, 4608], U16)

    # rate threshold: thr[4096:] = RN(65535 * (0.1*nz + (0.9*sig + 0.05)))
    # (conversion saturates both ends, which implements the reference clip)
    s09 = const.tile([128, 512], F32)
    nc.scalar.activation(s09[:], sig[:], AF.Copy, bias=0.05, scale=0.9)
    rnf = const.tile([128, 512], F32)
    nc.vector.scalar_tensor_tensor(rnf[:], nz[:], 0.1, s09[:], OP.mult, OP.add)
    nc.scalar.activation(thr[:, 4096:4608], rnf[:], AF.Copy, scale=65535.0)

    # temporal: st = floor(15*sig) = round(15*sig - 0.5), held in bf16 (exact)
    yt = const.tile([128, 512], F32)
    nc.scalar.activation(yt[:], sig[:], AF.Copy, scale=15.0, bias=-0.5)
    st = const.tile([128, 512], BF16)
    nc.vector.tensor_scalar(st[:], yt[:], MAGIC, MAGIC, OP.add, OP.subtract)

    # work pool opens BEFORE the scoped pw pool: stack-allocating pw on top
    # means rp prefetch tiles never sit in pw's region (whose reuse would
    # serialize their DMAs behind the resp matmuls).
    work = ctx.enter_context(tc.tile_pool(name="work", bufs=2))

    # hoist a few t-step loads ahead of the pw chunks in the SP HWDGE
    # FIFO so the pw stream can't fully starve the t-loop prefetch
    pre_rp = {}

    def prefetch_rp(t):
        rp = work.tile([128, 4608], U16, tag="rp", bufs=5, name=f"rp_pre{t}")
        nc.sync.dma_start(rp[:], aps["rpT"][t])
        pre_rp[t] = rp

    # population response: resp[p=(d_sub,n), (j,s)] for r-bank = x @ pop_w.T
    # fp16 weights and fp16 x in a single term (resp err ~3e-4 std, i.e.
    # ~1e-4 draw flips -- negligible). kc-outer so each pw chunk is consumed
    # as soon as its DMA lands.
    with tc.tile_pool(name="psum_r", bufs=1, space="PSUM") as psum_r, \
         tc.tile_pool(name="pwpool", bufs=2) as pwpool:
        banks = [psum_r.tile([128, 512], F32, name=f"bank{r}") for r in range(8)]
        for kc in range(4):
            p1k = pwpool.tile([128, 4096], F16, tag="p1k")
            nc.sync.dma_start(p1k[:], aps["pw1T"][:, kc * 4096:(kc + 1) * 4096])
            # interleave one rp prefetch per chunk: keeps DMA busy through the
            # (PE-paced) resp phase without delaying the pw chunks themselves
            prefetch_rp(kc)
            for r in range(8):
                for j in range(4):
                    co = (r * 4 + j) * 128
                    out = banks[r][:, j * 128:(j + 1) * 128]
                    # start zeroes the whole PSUM bank, so it must fire only
                    # on the very first matmul into each bank (kc==0, j==0)
                    nc.tensor.matmul(out, p1k[:, co:co + 128],
                                     xh[:, kc * 128:(kc + 1) * 128],
                                     start=(kc == 0 and j == 0),
                                     stop=(kc == 3 and j == 3))
        # evacuate: thr[0:4096] = RN(65535 * sigmoid(resp))
        prf = pwpool.tile([128, 4096], F32, tag="prf", bufs=1)
        for r in range(8):
            if add_pop_bias:
                for j in range(4):
                    nc.scalar.activation(
                        prf[:, r * 512 + j * 128:r * 512 + (j + 1) * 128],
                        banks[r][:, j * 128:(j + 1) * 128],
                        AF.Sigmoid, bias=pb[:, r * 4 + j:r * 4 + j + 1],
                    )
            else:
                nc.scalar.activation(prf[:, r * 512:(r + 1) * 512], banks[r][:], AF.Sigmoid)
        nc.vector.tensor_scalar(thr[:, 0:4096], prf[:], 65535.0, None, OP.mult)

    psum_o = ctx.enter_context(tc.tile_pool(name="psum_o", bufs=6, space="PSUM"))

    # ScalarE idles through the (PE/DMA-paced) resp phase; precompute the
    # sig-only temporal+phase encoders for the LAST timesteps there so the
    # end-of-kernel backlog shrinks to just the draw compare per step.
    PREENC = 9  # t >= PREENC precomputed

    def temporal_phase(t, sfx=""):
        """sig-only encoders for timestep t -> (oh, ph) bf16 tiles."""
        # temporal: one_hot(floor(15*sig))[t] == (st == t), holds w1
        oh = work.tile([128, 512], BF16, tag=f"oh{sfx}{t if sfx else ''}",
                       bufs=1 if sfx else 2, name=f"oh{t}{sfx}")
        nc.vector.tensor_scalar(oh[:], st[:], float(t), w1, OP.is_equal, OP.mult)

        # phase: q = (sig + FBF_t - 1)^2 per C-chunk, then (q-13/36)^2 > 1/9
        q = work.tile([128, 512], F32, tag="q", name=f"q{t}{sfx}")
        for c in range(4):
            nc.scalar.activation(
                q[:, c * 128:(c + 1) * 128], sig[:, c * 128:(c + 1) * 128],
                AF.Square, bias=fbq[:, t * 4 + c:t * 4 + c + 1],
            )
        q2 = work.tile([128, 512], F32, tag="q2", name=f"q2{t}{sfx}")
        nc.scalar.activation(q2[:], q[:], AF.Square, bias=c1336[:])
        ph = work.tile([128, 512], BF16, tag=f"ph{sfx}{t if sfx else ''}",
                       bufs=1 if sfx else 2, name=f"ph{t}{sfx}")
        nc.vector.tensor_scalar(ph[:], q2[:], 1.0 / 9.0, w3, OP.is_gt, OP.mult)
        return oh, ph

    def combine_store(t, rp, pre=None):
        """Fused draw compare + weighted 4-encoder combine in one PSUM bank."""
        cmp_ = work.tile([128, 4608], BF16, tag="cmp", bufs=2, name=f"cmp{t}")
        nc.vector.tensor_tensor(cmp_[:], rp[:], thr[:], OP.is_lt)
        oh, ph = pre if pre is not None else temporal_phase(t)
        O = psum_o.tile([128, 512], F32, tag="O", name=f"O{t}")
        nc.tensor.matmul(O[:], wI[:], cmp_[:, 4096:4608], start=True, stop=False)
        nc.tensor.matmul(O[:], iI[:], oh[:], start=False, stop=False)
        nc.tensor.matmul(O[:], iI[:], ph[:], start=False, stop=False)
        for r in range(8):
            nc.tensor.matmul(
                O[:], ew[:, r * 128:(r + 1) * 128], cmp_[:, r * 512:(r + 1) * 512],
                start=False, stop=(r == 7),
            )
        ot = work.tile([128, 512], out_dt, tag="ot", bufs=3, name=f"ot{t}")
        # with standard weights every output is k/32, k in [0,32]: ship as
        # uint8 = 32*out (exact; host divides back)
        nc.scalar.activation(ot[:], O[:], AF.Copy,
                             scale=32.0 if out_dt == U8 else 1.0)
        nc.scalar.dma_start(aps["outT"][t], ot[:])

    # ---------- per-timestep loop (fully unrolled) ----------
    PRESET = list(range(PREENC, T))
    pre_enc = {t: temporal_phase(t, sfx="P") for t in PRESET}
    for t in range(T - 1):
        if t in pre_rp:
            rp = pre_rp[t]
        else:
            rp = work.tile([128, 4608], U16, tag="rp", bufs=5)
            nc.sync.dma_start(rp[:], aps["rpT"][t])
        combine_store(t, rp, pre=pre_enc.get(t))

    # final timestep in chunks: each load chunk is compared (and its matmuls
    # run) while the next chunk streams, so the post-last-byte drain is one
    # chunk-compare + matmuls + store instead of the full chain
    tL = T - 1
    rp = work.tile([128, 4608], U16, tag="rp", bufs=5, name="rpL")
    cmpL = work.tile([128, 4608], BF16, tag="cmp", bufs=2, name="cmpL")
    ohL, phL = pre_enc[tL]
    O = psum_o.tile([128, 512], F32, tag="O", name="OL")
    nc.tensor.matmul(O[:], iI[:], ohL[:], start=True, stop=False)
    nc.tensor.matmul(O[:], iI[:], phL[:], start=False, stop=False)
    for q in range(4):
        lo = q * 1024
        nc.sync.dma_start(rp[:, lo:lo + 1024], aps["rpT"][tL, :, lo:lo + 1024])
        nc.vector.tensor_tensor(cmpL[:, lo:lo + 1024], rp[:, lo:lo + 1024],
                                thr[:, lo:lo + 1024], OP.is_lt)
        for r in (2 * q, 2 * q + 1):
            nc.tensor.matmul(
                O[:], ew[:, r * 128:(r + 1) * 128], cmpL[:, r * 512:(r + 1) * 512],
                start=False, stop=False)
    nc.sync.dma_start(rp[:, 4096:4608], aps["rpT"][tL, :, 4096:4608])
    nc.vector.tensor_tensor(cmpL[:, 4096:4608], rp[:, 4096:4608], thr[:, 4096:4608],
                            OP.is_lt)
    nc.tensor.matmul(O[:], wI[:], cmpL[:, 4096:4608], start=False, stop=True)
    ot = work.tile([128, 512], out_dt, tag="ot", bufs=3, name="otL")
    nc.scalar.activation(ot[:], O[:], AF.Copy,
                         scale=32.0 if out_dt == U8 else 1.0)
    nc.sync.dma_start(aps["outT"][tL], ot[:])


_CACHE = {}


def _out_is_k32(w):
    # outputs are sums of {w0,w1,w3}*{0,1} + w2*k/8; with the standard 0.25
    # weights every value is k/32, k in [0,32] -> exactly a scaled uint8
    return all(float(x) == 0.25 for x in w)


def _build(w, add_pop_bias):
    key = (tuple(float(x) for x in w), bool(add_pop_bias))
    if key in _CACHE:
        return _CACHE[key]
    out_dt = U8 if _out_is_k32(w) else F32
    nc = bacc.Bacc(
        "TRN2", target_bir_lowering=False, debug=False,
        enable_asserts=False, num_devices=NCORES,
    )
    aps = {}
    def di(name, shape, dt):
        aps[name] = nc.dram_tensor(name, shape, dt, kind="ExternalInput").ap()
    di("rpT", [T, 128, 4608], U16)
    di("pw1T", [128, 16384], F16)
    di("cfT", [128, 1089 + (32 if add_pop_bias else 0)], F32)
    di("cbT", [128, 1280], BF16)
    di("chT", [128, 512], F16)
    aps["outT"] = nc.dram_tensor("outT", [T, 128, 512], out_dt, kind="ExternalOutput").ap()

    with tile.TileContext(nc) as tc:
        _body(tc, aps, float(w[1]), float(w[3]), add_pop_bias, out_dt)
    nc.compile()
    _CACHE[key] = nc
    return nc


# ---------- host-side layout prep ----------

def _prep_ds(a):
    # [128 s, 512 d] -> [128 d', (C,s)]
    return np.ascontiguousarray(a.reshape(128, 4, 128).transpose(2, 1, 0)).reshape(128, 512)


def _prep_rr(a):
    # [T, 128 s, 512 d] -> [T, 128 d', (C,s)]
    return np.ascontiguousarray(a.reshape(T, 128, 4, 128).transpose(0, 3, 2, 1)).reshape(T, 128, 512)


def _prep_rp(a):
    # [T, 128 s, 512 d, 8 n] -> [T, 128 p=(d_sub,n), 4096 (r,j,s)]
    a6 = a.reshape(T, 128, 4, 8, 16, 8)  # t, s, j, r, d_sub, n
    return np.ascontiguousarray(a6.transpose(0, 4, 5, 3, 2, 1)).reshape(T, 128, 4096)


def _prep_pw(pwm):
    # [4096 e, 512 k] -> [128 k', 16384 (kc, r, j, p)]
    a = pwm.reshape(4, 8, 128, 4, 128)  # j, r, p, kc, k'
    return np.ascontiguousarray(a.transpose(4, 3, 1, 0, 2)).reshape(128, 16384)


def softmax_w(enc_weights):
    e = np.exp(enc_weights - enc_weights.max(), dtype=np.float32)
    return (e / e.sum(dtype=np.float32)).astype(np.float32)


def build_in_maps(inputs, w):
    x = np.asarray(inputs["x"], np.float32)
    freq_bands = np.asarray(inputs["freq_bands"], np.float32)
    pop_w = np.asarray(inputs["pop_w"], np.float32)
    pop_b = np.asarray(inputs["pop_b"], np.float32)
    noise_rate = np.asarray(inputs["noise_rate"], np.float32)
    rand_rate = np.asarray(inputs["rand_rate"], np.float32)
    rand_pop = np.asarray(inputs["rand_pop"], np.float32)
    add_pop_bias = bool(np.any(pop_b != 0))

    # shared constant tensors
    pw1T = _prep_pw(pop_w).astype(np.float16)
    # phase offsets: FBF - 1 = frac(t*f/15 - 1/4) - 1, per (t, C) bias columns
    # (reference arg is f*t_val + 2pi*sig with t_val = linspace(0, 2pi, T))
    t_vals = np.linspace(0.0, 2.0 * math.pi, T)  # float64, matches reference
    fq = freq_bands.reshape(4, 128).astype(np.float64)  # C, d'
    u_off = fq[None, :, :] * t_vals[:, None, None] / (2.0 * math.pi)  # [T, C, d']
    fbq = (np.mod(u_off - 0.25, 1.0) - 1.0).astype(np.float32)
    fbqT = np.ascontiguousarray(fbq.transpose(2, 0, 1)).reshape(128, T * 4)
    c1336 = np.full((128, 1), -13.0 / 36.0, np.float32)
    # ewm[p, (r, m)] = w2/8 where m = r*16 + p//8  (block-diag, zero-padded to
    # full 128 stationary columns so every matmul writes the whole PSUM bank)
    ewm = np.zeros((128, 8, 128), np.float32)
    for r in range(8):
        for ds in range(16):
            ewm[ds * 8:(ds + 1) * 8, r, r * 16 + ds] = w[2] / 8.0
    ewT = ewm.reshape(128, 1024).astype(ml_dtypes.bfloat16)
    w0I = (np.eye(128, dtype=np.float32) * w[0]).astype(ml_dtypes.bfloat16)
    I1 = np.eye(128, dtype=np.float32).astype(ml_dtypes.bfloat16)
    if add_pop_bias:
        # pb[p, r*4+j] = pop_b[e= j*1024 + r*128 + p]
        pbT = np.ascontiguousarray(
            pop_b.reshape(4, 8, 128).transpose(2, 1, 0)).reshape(128, 32)

    # draws -> uint16 fixed point (floor(65536*u), exact for u in [0,1))
    rp16 = np.minimum(np.floor(rand_pop.astype(np.float64) * 65536.0), 65535.0).astype(np.uint16)
    rr16 = np.minimum(np.floor(rand_rate.astype(np.float64) * 65536.0), 65535.0).astype(np.uint16)

    in_maps = []
    for c in range(NCORES):
        b, h = c // 2, c % 2
        sl = slice(h * SH, (h + 1) * SH)
        xs = x[b, sl]
        sigT = _prep_ds(1.0 / (1.0 + np.exp(-xs, dtype=np.float32)))
        cf_parts = [sigT, _prep_ds(noise_rate[b, sl]), fbqT, c1336]
        if add_pop_bias:
            cf_parts.append(pbT)
        m = {
            "rpT": np.concatenate(
                [_prep_rp(rp16[b, :, sl]), _prep_rr(rr16[b, :, sl])], axis=2),
            "pw1T": pw1T,
            "cfT": np.ascontiguousarray(np.concatenate(cf_parts, axis=1)),
            "cbT": np.ascontiguousarray(np.concatenate([ewT, w0I, I1], axis=1)),
            "chT": np.ascontiguousarray(_prep_ds(xs).astype(np.float16)),
        }
        in_maps.append(m)
    return in_maps


def kernel(x, freq_bands, pop_w, pop_b, enc_weights, noise_rate, rand_rate, rand_pop):
    inputs = dict(x=x, freq_bands=freq_bands, pop_w=pop_w, pop_b=pop_b,
                  enc_weights=enc_weights, noise_rate=noise_rate,
                  rand_rate=rand_rate, rand_pop=rand_pop)
    w = softmax_w(np.asarray(enc_weights, np.float32))
    add_pop_bias = bool(np.any(np.asarray(pop_b) != 0))
    nc = _build(w, add_pop_bias)
    in_maps = build_in_maps(inputs, w)

    res = run_bass_kernel_spmd(nc, in_maps, core_ids=list(range(NCORES)))

    out = np.empty((B, T, S, D), np.float32)
    for c in range(NCORES):
        b, h = c // 2, c % 2
        o = res.results[c]["outT"]  # [T, 128 d', (C,s)]
        if o.dtype == np.uint8:
            o = o.astype(np.float32) * np.float32(1.0 / 32.0)
        else:
            o = np.asarray(o, np.float32)
        o = o.reshape(T, 128, 4, 128).transpose(0, 3, 2, 1).reshape(T, SH, D)
        out[b, :, h * SH:(h + 1) * SH, :] = o
    return out


# revision 42
# speedup vs baseline: 1.6504x; 1.1525x over previous
"""AdvancedSpikeEncoder Trainium2 Bass kernel.

Sharding: 8 cores, core c handles batch b=c//2, seq-half h=c%2 (128 seq rows).
On-device layout is "transposed world": partition dim = d-slice, free dim = s.
  d = C*128 + d'      (C in [0,4), d' in [0,128))   for [128, (C,s)=512] tiles
  d = j*128 + r*16 + d_sub, p = d_sub*8 + n          for the population layout

DMA-minimizing dtypes (the kernel is HBM-bound):
  - Bernoulli draws ship as uint16 fixed point (floor(65536*u)); thresholds
    are built on device as RN(65535*p) uint16, so each compare is one 2-byte
    DVE op at the 2x rate and the quantization flip rate is ~1e-5.
  - pop_w and x ship fp16 (resp error ~3e-4 std, negligible vs draws).
  - sigmoid(x) ships f32: temporal/phase windows are sensitive to sig error
    (fp16 sig alone costs ~7e-3 rel), the 0.26MB is worth it.
  - output ships u8 (all outputs are exact multiples of 1/32 for the
    standard softmax weights; host divides back).

Encoders per timestep:
  - pop+rate: one fused [128,4608] uint16 is_lt against [prates16|r16].
  - temporal: st = floor(15*sig) once (round-trick, magic 1.5*2^23), then
    one 4x-mode is_equal per t.
  - phase: sin(f*t_val + 2pi*sig) > 0.5  <=>  |frac(u) - 1/4| < 1/6 with
    u = sig + t*f/15. With q = (sig + FBF_t - 1)^2 (FBF_t = frac-reduced
    offset, ScalarE Square with per-partition bias) the window test is
    (q - 13/36)^2 > 1/9: two ScalarE Squares + one DVE compare per t.
The weighted combine accumulates in one PSUM bank per timestep via matmuls
with (scaled-)identity stationaries; population mean over n rides on the
same bank via a block-diagonal stationary.
"""

import math
import os
import sys

import numpy as np

for _p in ("/opt/trn_rl_repo", "/root/.axon_site/_ro/trn_rl_repo"):
    if os.path.isdir(_p) and _p not in sys.path:
        sys.path.insert(0, _p)

import ml_dtypes  # noqa: E402

import concourse.bass as bass  # noqa: E402,F401
import concourse.tile as tile  # noqa: E402
from concourse import bacc, mybir  # noqa: E402
from concourse._compat import with_exitstack  # noqa: E402
from concourse.bass_utils import run_bass_kernel_spmd  # noqa: E402

B, T, S, D, N = 4, 16, 256, 512, 8
RP_BUFS = int(os.environ.get("K_RP_BUFS", "7"))
ENC_BUFS = int(os.environ.get("K_ENC_BUFS", "4"))  # unused on fused path
CMP_BUFS = int(os.environ.get("K_CMP_BUFS", "3"))
STORE_LAG = int(os.environ.get("K_STORE_LAG", "2"))
NCORES = 8
SH = 128  # seq rows per core
TWO_PI = float(2.0 * math.pi)
MAGIC = 12582912.0  # 1.5 * 2^23: round-to-nearest-int trick valid for |y| < 2^22

F32 = mybir.dt.float32
BF16 = mybir.dt.bfloat16
F16 = mybir.dt.float16
U16 = mybir.dt.uint16
U8 = mybir.dt.uint8


@with_exitstack
def _body(ctx, tc, aps, w1, w3, add_pop_bias, out_dt):
    nc = tc.nc
    AF = mybir.ActivationFunctionType
    OP = mybir.AluOpType

    const = ctx.enter_context(tc.tile_pool(name="const", bufs=1))

    # ---------- constants / preamble ----------
    # DMA issue order is by need-time: ch (x feeds the resp matmuls) goes
    # first; cf/cb are queued between pw chunks by the resp loop below.
    # fp16 blob: x 512 (for the resp matmul only)
    ch = const.tile([128, 512], F16, name="ch")
    nc.sync.dma_start(ch[:], aps["chT"][:])
    xh = ch[:, 0:512]
    # f32 blob: sig 512 | nz 512 | fbq 64 | c1336 1 | tmb 16 | c025 1 | cm19 1 | [pb 32]
    ncf = 1107 + (32 if add_pop_bias else 0)
    cf = const.tile([128, ncf], F32, name="cf")
    sig = cf[:, 0:512]
    nz = cf[:, 512:1024]
    fbq = cf[:, 1024:1088]
    c1336 = cf[:, 1088:1089]
    tmb = cf[:, 1089:1105]
    c025 = cf[:, 1105:1106]
    cm19 = cf[:, 1106:1107]
    if add_pop_bias:
        pb = cf[:, 1107:1139]
    # bf16 blob cols: ew 1024 | w0I 128 | I 128 | (w3/2)I 128
    cb = const.tile([128, 1408], BF16, name="cb")
    ew = cb[:, 0:1024]
    wI = cb[:, 1024:1152]
    iI = cb[:, 1152:1280]
    phI = cb[:, 1280:1408]

    # fused compare threshold [prates16 | r16], filled below
    thr = const.tile([128, 4608], U16)

    # work pool opens BEFORE the scoped pw pool: stack-allocating pw on top
    # means rp prefetch tiles never sit in pw's region (whose reuse would
    # serialize their DMAs behind the resp matmuls).
    work = ctx.enter_context(tc.tile_pool(name="work", bufs=2))

    # hoist a few t-step loads ahead of the pw chunks in the SP HWDGE
    # FIFO so the pw stream can't fully starve the t-loop prefetch
    pre_rp = {}

    def prefetch_rp(t):
        rp = work.tile([128, 4608], U16, tag="rp", bufs=RP_BUFS, name=f"rp_pre{t}")
        nc.sync.dma_start(rp[:], aps["rpT"][t])
        pre_rp[t] = rp

    # population response: resp[p=(d_sub,n), (j,s)] for r-bank = x @ pop_w.T
    # fp16 weights and fp16 x in a single term (resp err ~3e-4 std, i.e.
    # ~1e-4 draw flips -- negligible). kc-outer so each pw chunk is consumed
    # as soon as its DMA lands.
    with tc.tile_pool(name="psum_r", bufs=1, space="PSUM") as psum_r, \
         tc.tile_pool(name="pwpool", bufs=3) as pwpool:
        banks = [psum_r.tile([128, 512], F32, name=f"bank{r}") for r in range(8)]
        for kc in range(4):
            p1k = pwpool.tile([128, 4096], F16, tag="p1k")
            nc.sync.dma_start(p1k[:], aps["pw1T"][:, kc * 4096:(kc + 1) * 4096])
            # const blobs slot in behind the first chunks (needed ~when the
            # evacuation runs); rp prefetches go after all pw chunks
            if kc == 0:
                nc.sync.dma_start(cf[:], aps["cfT"][:])
            elif kc == 1:
                nc.sync.dma_start(cb[:], aps["cbT"][:])
            for r in range(8):
                for j in range(4):
                    co = (r * 4 + j) * 128
                    out = banks[r][:, j * 128:(j + 1) * 128]
                    # start zeroes the whole PSUM bank, so it must fire only
                    # on the very first matmul into each bank (kc==0, j==0)
                    nc.tensor.matmul(out, p1k[:, co:co + 128],
                                     xh[:, kc * 128:(kc + 1) * 128],
                                     start=(kc == 0 and j == 0),
                                     stop=(kc == 3 and j == 3))
        # cf-dependent preamble compute goes AFTER the cf DMA was issued
        # (the tile tracker needs writer-before-reader in issue order).
        # rate threshold: thr[4096:] = RN(65535*(0.1*nz + (0.9*sig + 0.05)));
        # the saturating conversion implements the reference clip
        s09 = const.tile([128, 512], F32)
        nc.scalar.activation(s09[:], sig[:], AF.Copy, bias=0.05, scale=0.9)
        rnf = const.tile([128, 512], F32)
        nc.vector.scalar_tensor_tensor(rnf[:], nz[:], 0.1, s09[:], OP.mult, OP.add)
        nc.scalar.activation(thr[:, 4096:4608], rnf[:], AF.Copy, scale=65535.0)
        # temporal: st = floor(15*sig) = round(15*sig - 0.5), bf16 (exact)
        yt = const.tile([128, 512], F32)
        nc.scalar.activation(yt[:], sig[:], AF.Copy, scale=15.0, bias=-0.5)
        st = const.tile([128, 512], BF16)
        nc.vector.tensor_scalar(st[:], yt[:], MAGIC, MAGIC, OP.add, OP.subtract)

        # evacuate: thr[0:4096] = RN(65535 * sigmoid(resp))
        prf = pwpool.tile([128, 4096], F32, tag="prf", bufs=1)
        for r in range(8):
            if add_pop_bias:
                for j in range(4):
                    nc.scalar.activation(
                        prf[:, r * 512 + j * 128:r * 512 + (j + 1) * 128],
                        banks[r][:, j * 128:(j + 1) * 128],
                        AF.Sigmoid, bias=pb[:, r * 4 + j:r * 4 + j + 1],
                    )
            else:
                nc.scalar.activation(prf[:, r * 512:(r + 1) * 512], banks[r][:], AF.Sigmoid)
        nc.vector.tensor_scalar(thr[:, 0:4096], prf[:], 65535.0, None, OP.mult)
    # prefetch the first t-tiles once the pw stream is fully queued
    for t in range(RP_BUFS):
        prefetch_rp(t)

    psum_o = ctx.enter_context(tc.tile_pool(name="psum_o", bufs=6, space="PSUM"))

    # ScalarE/DVE idle through the (PE/DMA-paced) resp phase; precompute the
    # sig-only temporal+phase encoders for the LAST timesteps there (fused
    # into one tile = one PE matmul); early timesteps compute them in-loop
    # as separate tiles (two matmuls, no DVE fold).
    PREENC = int(os.environ.get("K_PREENC", "8"))

    # temporal tiles depend only on st: all 16 are built in the resp window
    oh_pre = {}
    for _t in range(T):
        _oh = work.tile([128, 512], BF16, tag=f"ohp{_t}", bufs=1, name=f"ohp{_t}")
        nc.vector.tensor_scalar(_oh[:], st[:], float(_t), w1, OP.is_equal, OP.mult)
        oh_pre[_t] = _oh

    def temporal_phase(t, fuse, sfx=""):
        """phase window (+ fused temporal) for timestep t."""
        oh = oh_pre[t]

        # phase: q = (sig + FBF_t - 1)^2 per C-chunk; (q-13/36)^2 > 1/9
        q = work.tile([128, 512], F32, tag="q", name=f"q{t}{sfx}")
        for c in range(4):
            nc.scalar.activation(
                q[:, c * 128:(c + 1) * 128], sig[:, c * 128:(c + 1) * 128],
                AF.Square, bias=fbq[:, t * 4 + c:t * 4 + c + 1],
            )
        q2 = work.tile([128, 512], F32, tag="q2", name=f"q2{t}{sfx}")
        nc.scalar.activation(q2[:], q[:], AF.Square, bias=c1336[:])
        ph = work.tile([128, 512], BF16, tag="phs" if fuse else f"phi{t % 2}",
                       bufs=2 if fuse else 1, name=f"ph{t}{sfx}")
        nc.vector.tensor_scalar(ph[:], q2[:], 1.0 / 9.0, w3, OP.is_gt, OP.mult)
        if not fuse:
            return oh, ph
        op = work.tile([128, 512], BF16, tag=f"op{t}", bufs=1, name=f"op{t}{sfx}")
        nc.vector.tensor_tensor(op[:], oh[:], ph[:], OP.add)
        return op

    evac_scale = 32.0 if out_dt == U8 else 1.0
    evac_bias = 0.0

    def combine_store(t, rp, pre=None):
        """Fused draw compare + weighted 4-encoder combine in one PSUM bank."""
        cmp_ = work.tile([128, 4608], BF16, tag="cmp", bufs=CMP_BUFS, name=f"cmp{t}")
        nc.vector.tensor_tensor(cmp_[:], rp[:], thr[:], OP.is_lt)
        enc = pre if pre is not None else temporal_phase(t, fuse=False)
        O = psum_o.tile([128, 512], F32, tag="O", name=f"O{t}")
        nc.tensor.matmul(O[:], wI[:], cmp_[:, 4096:4608], start=True, stop=False)
        if pre is not None:
            nc.tensor.matmul(O[:], iI[:], enc[:], start=False, stop=False)
        else:
            oh, ph = enc
            nc.tensor.matmul(O[:], iI[:], oh[:], start=False, stop=False)
            nc.tensor.matmul(O[:], iI[:], ph[:], start=False, stop=False)
        for r in range(8):
            nc.tensor.matmul(
                O[:], ew[:, r * 128:(r + 1) * 128], cmp_[:, r * 512:(r + 1) * 512],
                start=False, stop=(r == 7),
            )
        ot = work.tile([128, 512], out_dt, tag="ot", bufs=4, name=f"ot{t}")
        # with standard weights every output is k/32, k in [0,32]: ship as
        # uint8 = 32*out (exact; host divides back)
        nc.scalar.activation(ot[:], O[:], AF.Copy, scale=evac_scale,
                             bias=evac_bias)
        pend_store.append((t, ot))
        if len(pend_store) > STORE_LAG:
            tp, otp = pend_store.pop(0)
            nc.scalar.dma_start(aps["outT"][tp], otp[:])

    # ---------- per-timestep loop (fully unrolled) ----------
    pend_store = []
    pre_enc = {t: temporal_phase(t, fuse=True, sfx="P") for t in range(PREENC, T)}
    for t in range(T - 1):
        if t in pre_rp:
            rp = pre_rp[t]
        else:
            rp = work.tile([128, 4608], U16, tag="rp", bufs=RP_BUFS)
            nc.sync.dma_start(rp[:], aps["rpT"][t])
        combine_store(t, rp, pre=pre_enc.get(t))

    # final timestep in chunks: each load chunk is compared (and its matmuls
    # run) while the next chunk streams, so the post-last-byte drain is one
    # chunk-compare + matmuls + store instead of the full chain
    tL = T - 1
    rp = work.tile([128, 4608], U16, tag="rp", bufs=RP_BUFS, name="rpL")
    cmpL = work.tile([128, 4608], BF16, tag="cmp", bufs=CMP_BUFS, name="cmpL")
    opL = pre_enc[tL]
    O = psum_o.tile([128, 512], F32, tag="O", name="OL")
    nc.tensor.matmul(O[:], iI[:], opL[:], start=True, stop=False)
    for q in range(4):
        lo = q * 1024
        nc.sync.dma_start(rp[:, lo:lo + 1024], aps["rpT"][tL, :, lo:lo + 1024])
        nc.vector.tensor_tensor(cmpL[:, lo:lo + 1024], rp[:, lo:lo + 1024],
                                thr[:, lo:lo + 1024], OP.is_lt)
        for r in (2 * q, 2 * q + 1):
            nc.tensor.matmul(
                O[:], ew[:, r * 128:(r + 1) * 128], cmpL[:, r * 512:(r + 1) * 512],
                start=False, stop=False)
    nc.sync.dma_start(rp[:, 4096:4608], aps["rpT"][tL, :, 4096:4608])
    nc.vector.tensor_tensor(cmpL[:, 4096:4608], rp[:, 4096:4608], thr[:, 4096:4608],
                            OP.is_lt)
    nc.tensor.matmul(O[:], wI[:], cmpL[:, 4096:4608], start=False, stop=True)
    for tp, otp in pend_store:
        nc.scalar.dma_start(aps["outT"][tp], otp[:])
    ot = work.tile([128, 512], out_dt, tag="ot", bufs=4, name="otL")
    nc.scalar.activation(ot[:], O[:], AF.Copy, scale=evac_scale, bias=evac_bias)
    nc.sync.dma_start(aps["outT"][tL], ot[:])


_CACHE = {}


def _out_is_k32(w):
    # outputs are sums of {w0,w1,w3}*{0,1} + w2*k/8; with the standard 0.25
    # weights every value is k/32, k in [0,32] -> exactly a scaled uint8
    return all(float(x) == 0.25 for x in w)


def _build(w, add_pop_bias):
    key = (tuple(float(x) for x in w), bool(add_pop_bias))
    if key in _CACHE:
        return _CACHE[key]
    out_dt = U8 if _out_is_k32(w) else F32
    nc = bacc.Bacc(
        "TRN2", target_bir_lowering=False, debug=False,
        enable_asserts=False, num_devices=NCORES,
    )
    aps = {}
    def di(name, shape, dt):
        aps[name] = nc.dram_tensor(name, shape, dt, kind="ExternalInput").ap()
    di("rpT", [T, 128, 4608], U16)
    di("pw1T", [128, 16384], F16)
    di("cfT", [128, 1107 + (32 if add_pop_bias else 0)], F32)
    di("cbT", [128, 1408], BF16)
    di("chT", [128, 512], F16)
    aps["outT"] = nc.dram_tensor("outT", [T, 128, 512], out_dt, kind="ExternalOutput").ap()

    with tile.TileContext(nc) as tc:
        _body(tc, aps, float(w[1]), float(w[3]), add_pop_bias, out_dt)
    nc.compile()
    _CACHE[key] = nc
    return nc


# ---------- host-side layout prep ----------

def _prep_ds(a):
    # [128 s, 512 d] -> [128 d', (C,s)]
    return np.ascontiguousarray(a.reshape(128, 4, 128).transpose(2, 1, 0)).reshape(128, 512)


def _prep_rr(a):
    # [T, 128 s, 512 d] -> [T, 128 d', (C,s)]
    return np.ascontiguousarray(a.reshape(T, 128, 4, 128).transpose(0, 3, 2, 1)).reshape(T, 128, 512)


def _prep_rp(a):
    # [T, 128 s, 512 d, 8 n] -> [T, 128 p=(d_sub,n), 4096 (r,j,s)]
    a6 = a.reshape(T, 128, 4, 8, 16, 8)  # t, s, j, r, d_sub, n
    return np.ascontiguousarray(a6.transpose(0, 4, 5, 3, 2, 1)).reshape(T, 128, 4096)


def _prep_pw(pwm):
    # [4096 e, 512 k] -> [128 k', 16384 (kc, r, j, p)]
    a = pwm.reshape(4, 8, 128, 4, 128)  # j, r, p, kc, k'
    return np.ascontiguousarray(a.transpose(4, 3, 1, 0, 2)).reshape(128, 16384)


def softmax_w(enc_weights):
    e = np.exp(enc_weights - enc_weights.max(), dtype=np.float32)
    return (e / e.sum(dtype=np.float32)).astype(np.float32)


def build_in_maps(inputs, w):
    x = np.asarray(inputs["x"], np.float32)
    freq_bands = np.asarray(inputs["freq_bands"], np.float32)
    pop_w = np.asarray(inputs["pop_w"], np.float32)
    pop_b = np.asarray(inputs["pop_b"], np.float32)
    noise_rate = np.asarray(inputs["noise_rate"], np.float32)
    rand_rate = np.asarray(inputs["rand_rate"], np.float32)
    rand_pop = np.asarray(inputs["rand_pop"], np.float32)
    add_pop_bias = bool(np.any(pop_b != 0))

    # shared constant tensors
    pw1T = _prep_pw(pop_w).astype(np.float16)
    # phase offsets: FBF - 1 = frac(t*f/15 - 1/4) - 1, per (t, C) bias columns
    # (reference arg is f*t_val + 2pi*sig with t_val = linspace(0, 2pi, T))
    t_vals = np.linspace(0.0, 2.0 * math.pi, T)  # float64, matches reference
    fq = freq_bands.reshape(4, 128).astype(np.float64)  # C, d'
    u_off = fq[None, :, :] * t_vals[:, None, None] / (2.0 * math.pi)  # [T, C, d']
    fbq = (np.mod(u_off - 0.25, 1.0) - 1.0).astype(np.float32)
    fbqT = np.ascontiguousarray(fbq.transpose(2, 0, 1)).reshape(128, T * 4)
    c1336 = np.full((128, 1), -13.0 / 36.0, np.float32)
    tmb = np.broadcast_to(-np.arange(T, dtype=np.float32), (128, T)).copy()
    c025 = np.full((128, 1), 0.25, np.float32)
    cm19 = np.full((128, 1), -1.0 / 9.0, np.float32)
    # ewm[p, (r, m)] = w2/8 where m = r*16 + p//8  (block-diag, zero-padded to
    # full 128 stationary columns so every matmul writes the whole PSUM bank)
    ewm = np.zeros((128, 8, 128), np.float32)
    for r in range(8):
        for ds in range(16):
            ewm[ds * 8:(ds + 1) * 8, r, r * 16 + ds] = w[2] / 8.0
    ewT = ewm.reshape(128, 1024).astype(ml_dtypes.bfloat16)
    eye = np.eye(128, dtype=np.float32)
    w0I = (eye * w[0]).astype(ml_dtypes.bfloat16)
    I1 = eye.astype(ml_dtypes.bfloat16)
    w3I2 = (eye * (w[3] * 0.5)).astype(ml_dtypes.bfloat16)
    if add_pop_bias:
        # pb[p, r*4+j] = pop_b[e= j*1024 + r*128 + p]
        pbT = np.ascontiguousarray(
            pop_b.reshape(4, 8, 128).transpose(2, 1, 0)).reshape(128, 32)

    # draws -> uint16 fixed point (floor(65536*u), exact for u in [0,1))
    rp16 = np.minimum(np.floor(rand_pop.astype(np.float64) * 65536.0), 65535.0).astype(np.uint16)
    rr16 = np.minimum(np.floor(rand_rate.astype(np.float64) * 65536.0), 65535.0).astype(np.uint16)

    in_maps = []
    for c in range(NCORES):
        b, h = c // 2, c % 2
        sl = slice(h * SH, (h + 1) * SH)
        xs = x[b, sl]
        sigT = _prep_ds(1.0 / (1.0 + np.exp(-xs, dtype=np.float32)))
        cf_parts = [sigT, _prep_ds(noise_rate[b, sl]), fbqT, c1336, tmb, c025, cm19]
        if add_pop_bias:
            cf_parts.append(pbT)
        m = {
            "rpT": np.concatenate(
                [_prep_rp(rp16[b, :, sl]), _prep_rr(rr16[b, :, sl])], axis=2),
            "pw1T": pw1T,
            "cfT": np.ascontiguousarray(np.concatenate(cf_parts, axis=1)),
            "cbT": np.ascontiguousarray(np.concatenate([ewT, w0I, I1, w3I2], axis=1)),
            "chT": np.ascontiguousarray(_prep_ds(xs).astype(np.float16)),
        }
        in_maps.append(m)
    return in_maps


def kernel(x, freq_bands, pop_w, pop_b, enc_weights, noise_rate, rand_rate, rand_pop):
    inputs = dict(x=x, freq_bands=freq_bands, pop_w=pop_w, pop_b=pop_b,
                  enc_weights=enc_weights, noise_rate=noise_rate,
                  rand_rate=rand_rate, rand_pop=rand_pop)
    w = softmax_w(np.asarray(enc_weights, np.float32))
    add_pop_bias = bool(np.any(np.asarray(pop_b) != 0))
    nc = _build(w, add_pop_bias)
    in_maps = build_in_maps(inputs, w)

    res = run_bass_kernel_spmd(nc, in_maps, core_ids=list(range(NCORES)))

    out = np.empty((B, T, S, D), np.float32)
    for c in range(NCORES):
        b, h = c // 2, c % 2
        o = res.results[c]["outT"]  # [T, 128 d', (C,s)]
        if o.dtype == np.uint8:
            o = o.astype(np.float32) * np.float32(1.0 / 32.0)
        else:
            o = np.asarray(o, np.float32)
        o = o.reshape(T, 128, 4, 128).transpose(0, 3, 2, 1).reshape(T, SH, D)
        out[b, :, h * SH:(h + 1) * SH, :] = o
    return out
